# revision 1
# baseline (speedup 1.0000x reference)
"""DCT-based 1.25x upsample (2D DCT-II -> zero-pad spectrum -> 2D IDCT).

The whole reference computation is linear per (b, c) slice:
    out = M @ x @ M^T,   M = E960[:, :768] @ D768  (960x768, precomputed on host)
where D is the DCT-II matrix (norm=None) and E the IDCT matrix; zero-padding
the spectrum is folded into taking the first 768 columns of E.

On each NeuronCore (data-parallel over batch: 16 batches / 8 cores = 2 each,
x 3 channels = 6 slices per core) we run two chained matmuls per slice with
the tensor engine's `out = lhsT.T @ rhs` convention and the shared constant
Mt = M^T (768x960) as the moving operand:
    step 1:  W   = matmul(lhsT=x,  rhs=Mt) = x^T @ M^T         [768, 960]
    step 2:  out = matmul(lhsT=W,  rhs=Mt) = M @ x @ M^T       [960, 960]
W comes out of step 1 in PSUM with exactly the [K-partition, free] layout
step 2 needs for lhsT, so no transposes anywhere.

Matmuls run as float32r (fp32 bits, FP22 multiplies, fp32 accumulate):
1 PE cycle/row vs 4 for true fp32, end-to-end rel err ~1e-4.
"""

import numpy as np

import concourse.bass as bass  # noqa: F401  (engine types route via nc)
import concourse.mybir as mybir
import concourse.tile as tile
from concourse import bacc
from concourse.bass_utils import run_bass_kernel_spmd

# Problem shape (hardcoded per contract)
B, C, H = 16, 3, 768
OUT = 960  # H * 1.25
N_CORES = 8
SLICES = (B * C) // N_CORES  # 6 per core

P = 128
KT = H // P          # 6 contraction tiles
MT1 = H // P         # 6 output-row tiles for step 1 (x columns)
M2 = 120
MT2 = OUT // M2      # 8 output-row tiles for step 2
NT = 2
NW = OUT // NT       # 480-wide moving chunks (<= 512 fp32 PSUM bank)

MM_DT = mybir.dt.float32r  # set to mybir.dt.float32 for full-precision (4x slower)


def _build_mt() -> np.ndarray:
    """Mt = (E960[:, :768] @ D768)^T as float32, computed in float64."""
    n = np.arange(H, dtype=np.float64)
    k = np.arange(H, dtype=np.float64)[:, None]
    D = 2.0 * np.cos(np.pi * (2.0 * n[None, :] + 1.0) * k / (2.0 * H))

    n2 = np.arange(OUT, dtype=np.float64)[:, None]
    k2 = np.arange(OUT, dtype=np.float64)[None, :]
    E = np.cos(np.pi * (2.0 * n2 + 1.0) * k2 / (2.0 * OUT)) / OUT
    E[:, 0] = 1.0 / (2.0 * OUT)

    M = E[:, :H] @ D  # [960, 768]
    return np.ascontiguousarray(M.T).astype(np.float32)  # [768, 960]


def _build_program():
    nc = bacc.Bacc(None, target_bir_lowering=False, debug=False)

    # Both inputs are pre-arranged on the host into the striped SBUF layout
    # (partition-major), so every load DMA is one contiguous run per partition.
    x_ext = nc.dram_tensor("x", [SLICES, P, KT * H], MM_DT, kind="ExternalInput")
    mt_ext = nc.dram_tensor("mt", [P, KT * OUT], MM_DT, kind="ExternalInput")
    out_ext = nc.dram_tensor(
        "out", [SLICES, OUT, OUT], mybir.dt.float32, kind="ExternalOutput"
    )

    with tile.TileContext(nc) as tc:
        with (
            tc.tile_pool(name="const", bufs=1) as const_pool,
            tc.tile_pool(name="xp", bufs=4) as x_pool,
            tc.tile_pool(name="wp", bufs=2) as w_pool,
            tc.tile_pool(name="op", bufs=4) as o_pool,
            tc.tile_pool(name="ps", bufs=8, space="PSUM") as psum_pool,
        ):
            # PE warmup: dummy matmuls on memset tiles keep the tensor engine
            # busy while the first loads land, so the HAM clock gate is already
            # at 2.4 GHz (not the idle 1.2 GHz) when real matmuls start.
            warm_f32 = const_pool.tile([P, NW], mybir.dt.float32, name="warm_f32")
            nc.gpsimd.memset(warm_f32[:], 0.0)
            warm_w = const_pool.tile([P, P], MM_DT, name="warm_w")
            warm_m = const_pool.tile([P, NW], MM_DT, name="warm_m")
            nc.vector.tensor_copy(warm_w[:], warm_f32[:, :P])
            nc.vector.tensor_copy(warm_m[:], warm_f32[:])
            warm_ps = psum_pool.tile([P, NW], mybir.dt.float32, tag="ps", name="warm_ps")
            for _ in range(36):
                nc.tensor.matmul(warm_ps[:], warm_w[:], warm_m[:], start=True, stop=True)

            # Shared constant Mt split into two K-group tiles (contiguous DRAM
            # runs per partition); the first matmuls only wait on group 0.
            # mt_kg[g][p, kl, n] = Mt[(g*KH + kl)*P + p, n]
            KH = KT // 2
            mt_dram = mt_ext[:].rearrange("p (ko n) -> p ko n", n=OUT)
            mt_kg = [
                const_pool.tile([P, KH, OUT], MM_DT, name=f"mt{g}") for g in range(2)
            ]
            nc.sync.dma_start(mt_kg[0][:], mt_dram[:, 0:KH, :])

            for s in range(SLICES):
                # x slice split into two K-group (row) tiles:
                # x_kg[g][p, kl, j] = x[(g*KH + kl)*P + p, j]
                x_dram = x_ext[s].rearrange("p (ko j) -> p ko j", j=H)
                x_kg = []
                for g in range(2):
                    xg = x_pool.tile([P, KH, H], MM_DT, tag="x", name=f"x{g}")
                    nc.sync.dma_start(xg[:], x_dram[:, g * KH : (g + 1) * KH, :])
                    x_kg.append(xg)
                    if s == 0 and g == 0:
                        nc.sync.dma_start(mt_kg[1][:], mt_dram[:, KH:KT, :])

                # Step 1: W = x^T @ Mt, K-striped for step 2:
                # w_sb[p, m, l] = W[m*P + p, l]
                w_sb = w_pool.tile([P, KT, OUT], MM_DT)
                MH = MT1 // 2
                if s == 0:
                    # Two-pass accumulation, K-group outer: all of group 0's
                    # matmuls run while group 1's loads are still in flight;
                    # partials stash in SBUF and fold back in during group 1.
                    w_part = w_pool.tile(
                        [P, KT, OUT], mybir.dt.float32, name="w_part", bufs=1
                    )
                    for g in range(2):
                        for half in range(2):
                            psums = [
                                [
                                    psum_pool.tile(
                                        [P, NW],
                                        mybir.dt.float32,
                                        tag="ps",
                                        name=f"ps{ml}_{n}",
                                    )
                                    for n in range(NT)
                                ]
                                for ml in range(MH)
                            ]
                            for kl in range(KH):
                                for ml in range(MH):
                                    m = half * MH + ml
                                    for n in range(NT):
                                        nc.tensor.matmul(
                                            psums[ml][n][:],
                                            x_kg[g][:, kl, m * P : (m + 1) * P],
                                            mt_kg[g][:, kl, n * NW : (n + 1) * NW],
                                            start=(kl == 0),
                                            stop=(kl == KH - 1),
                                        )
                            for ml in range(MH):
                                m = half * MH + ml
                                for n in range(NT):
                                    dst = slice(n * NW, (n + 1) * NW)
                                    if g == 0:
                                        nc.vector.tensor_copy(
                                            w_part[:, m, dst], psums[ml][n][:]
                                        )
                                    else:
                                        nc.vector.tensor_add(
                                            out=w_sb[:, m, dst],
                                            in0=psums[ml][n][:],
                                            in1=w_part[:, m, dst],
                                        )
                else:
                    for half in range(2):
                        psums = [
                            [
                                psum_pool.tile(
                                    [P, NW],
                                    mybir.dt.float32,
                                    tag="ps",
                                    name=f"ps{ml}_{n}",
                                )
                                for n in range(NT)
                            ]
                            for ml in range(MH)
                        ]
                        for k in range(KT):
                            g, kl = divmod(k, KH)
                            for ml in range(MH):
                                m = half * MH + ml
                                for n in range(NT):
                                    nc.tensor.matmul(
                                        psums[ml][n][:],
                                        x_kg[g][:, kl, m * P : (m + 1) * P],
                                        mt_kg[g][:, kl, n * NW : (n + 1) * NW],
                                        start=(k == 0),
                                        stop=(k == KT - 1),
                                    )
                        for ml in range(MH):
                            m = half * MH + ml
                            for n in range(NT):
                                nc.vector.tensor_copy(
                                    w_sb[:, m, n * NW : (n + 1) * NW], psums[ml][n][:]
                                )
                w_r = w_sb[:]

                # Step 2: out = W^T @ Mt
                for m in range(MT2):
                    psums = [
                        psum_pool.tile([P, NW], mybir.dt.float32, tag="ps", name=f"ps{n}")
                        for n in range(NT)
                    ]
                    o_sb = o_pool.tile([M2, OUT], mybir.dt.float32)
                    for k in range(KT):
                        g, kl = divmod(k, KH)
                        for n in range(NT):
                            nc.tensor.matmul(
                                psums[n][:M2, :],
                                w_r[:, k, m * M2 : (m + 1) * M2],
                                mt_kg[g][:, kl, n * NW : (n + 1) * NW],
                                start=(k == 0),
                                stop=(k == KT - 1),
                            )
                    for n in range(NT):
                        nc.vector.tensor_copy(
                            o_sb[:, n * NW : (n + 1) * NW], psums[n][:M2, :]
                        )
                        nc.sync.dma_start(
                            out_ext[s, m * M2 : (m + 1) * M2, n * NW : (n + 1) * NW],
                            o_sb[:, n * NW : (n + 1) * NW],
                        )

    nc.compile()
    return nc


_CACHE: dict = {}


def _get_program():
    if "nc" not in _CACHE:
        _CACHE["nc"] = _build_program()
        _CACHE["mt"] = _build_mt()
    return _CACHE["nc"], _CACHE["mt"]


def kernel(x: np.ndarray, _trace: bool = False):
    assert x.shape == (B, C, H, H), x.shape
    nc, mt = _get_program()
    x = np.ascontiguousarray(x, dtype=np.float32)
    # Pre-stripe on host: rows -> (ko, p) partitions, contiguous per partition.
    mt_arr = np.ascontiguousarray(
        mt.reshape(KT, P, OUT).transpose(1, 0, 2).reshape(P, KT * OUT)
    )
    x_arr = np.ascontiguousarray(
        x.reshape(B * C, KT, P, H).transpose(0, 2, 1, 3).reshape(B * C, P, KT * H)
    )
    per_core = B // N_CORES
    in_maps = [
        {
            "x": x_arr[i * SLICES : (i + 1) * SLICES],
            "mt": mt_arr,
        }
        for i in range(N_CORES)
    ]
    res = run_bass_kernel_spmd(nc, in_maps, list(range(N_CORES)), trace=_trace)
    out = np.empty((B, C, OUT, OUT), dtype=np.float32)
    for i in range(N_CORES):
        out[i * per_core : (i + 1) * per_core] = res.results[i]["out"].reshape(
            per_core, C, OUT, OUT
        )
    if _trace:
        return out, res
    return out



# revision 3
# speedup vs baseline: 1.8521x; 1.8521x over previous
"""DCT-based 1.25x upsample (2D DCT-II -> zero-pad spectrum -> 2D IDCT).

The reference computation is linear per (b, c) slice: out = M @ x @ M^T with
M = E960[:, :768] @ D768 (960x768). M is *centrosymmetric*
(M[959-i, 767-n] = M[i, n], from the DCT even/odd symmetry), so the classic
symmetric/antisymmetric fold halves the matmul FLOPs:

    MP = (M[:480, :384] + M[:480, 767:383:-1]) / 2     [480, 384]
    MM = (M[:480, :384] - M[:480, 767:383:-1]) / 2
    x_pq = (row-fold p)(col-fold q)(x)                 4 tiles of [384, 384]

    P1 = x_pp MP^T   P2 = x_pm MM^T   P3 = x_mp MP^T   P4 = x_mm MM^T
    A = P1+P2  C = P1-P2  B = P3+P4  D = P3-P4         [384, 480] each
    t_tl = MP A + MM B    t_bl = MP A - MM B           [480, 480] each
    t_tr = MP C + MM D    t_br = MP C - MM D

    out = [[t_tl, fliplr(t_tr)], [flipud(t_bl), flipud(fliplr(t_br))]]

Input folds and output quadrant assembly are O(N^2) passes done on the host
(like the layout striping); the device runs only the half-FLOP matmuls plus
the +- combines. Everything on device is bf16 (errors ~0.3% << 2e-2 gate):
matmuls run at 1 PE cycle/row, DMA traffic halves, and the SBUF-side DVE
combines hit the 2x packed perf mode.

Engine split per slice (all overlapped, tensor-bound):
  PE:      84 matmuls (N=480, bf16)                    ~17.0 us
  DVE:     stage-1 +- combines from PSUM (12 ops) and
           stage-2 +- combines from SBUF bf16 (8 ops)  ~12.5 us
  ScalarE: stage-2 PSUM->SBUF bf16 copies (8 ops)      ~8.7 us
  SP/DMA:  1 input + 8 output DMAs                     ~9.5 us
"""

import numpy as np
import ml_dtypes

import concourse.bass as bass  # noqa: F401  (engine types route via nc)
import concourse.mybir as mybir
import concourse.tile as tile
from concourse import bacc
from concourse.bass_utils import run_bass_kernel_spmd

# Problem shape (hardcoded per contract)
B, C, H = 16, 3, 768
OUT = 960  # H * 1.25
N_CORES = 8
SLICES = (B * C) // N_CORES  # 6 per core

P = 128
HF = H // 2    # 384: folded input length
QF = OUT // 2  # 480: folded output length
KT = HF // P   # 3 contraction tiles of 128
NT1 = HF // P  # 3 stage-1 output-row tiles (row-fold index)
M2 = 120
MT2 = QF // M2  # 4 stage-2 output-row tiles

DT = mybir.dt.bfloat16
BF16 = ml_dtypes.bfloat16


def _build_consts() -> np.ndarray:
    """[128, 2*KT*QF] bf16: MPt/MMt = MP^T/MM^T striped K -> (kl, p)."""
    n = np.arange(H, dtype=np.float64)
    k = np.arange(H, dtype=np.float64)[:, None]
    D = 2.0 * np.cos(np.pi * (2.0 * n[None, :] + 1.0) * k / (2.0 * H))

    n2 = np.arange(OUT, dtype=np.float64)[:, None]
    k2 = np.arange(OUT, dtype=np.float64)[None, :]
    E = np.cos(np.pi * (2.0 * n2 + 1.0) * k2 / (2.0 * OUT)) / OUT
    E[:, 0] = 1.0 / (2.0 * OUT)

    M = E[:, :H] @ D  # [960, 768]
    MP = (M[:QF, :HF] + M[:QF, H - 1 : HF - 1 : -1]) / 2.0  # [480, 384]
    MM = (M[:QF, :HF] - M[:QF, H - 1 : HF - 1 : -1]) / 2.0
    mt = np.stack([MP.T, MM.T])  # [2, 384, 480], K first
    # K -> (kl, p) striping, partition-major: [128, 2, KT, QF]
    mt = mt.reshape(2, KT, P, QF).transpose(2, 0, 1, 3)
    return np.ascontiguousarray(mt.reshape(P, 2 * KT * QF)).astype(BF16)


def _fold_inputs(x: np.ndarray) -> np.ndarray:
    """Host fold + lhsT striping: [B*C, 128, 4*KT*HF] bf16.

    Product order t=0..3 pairs (x_pp, MPt), (x_pm, MMt), (x_mp, MPt),
    (x_mm, MMt). lhsT layout per slice: x_sb[p, t, kl, n] = x_t[n, kl*128+p]
    (K = col-fold index m on partitions, M = row-fold index n free).
    """
    xr = x.reshape(B * C, H, H)
    fp = xr[:, :, :HF] + xr[:, :, H - 1 : HF - 1 : -1]  # col fold
    fm = xr[:, :, :HF] - xr[:, :, H - 1 : HF - 1 : -1]
    xpp = fp[:, :HF] + fp[:, H - 1 : HF - 1 : -1]  # row fold
    xmp = fp[:, :HF] - fp[:, H - 1 : HF - 1 : -1]
    xpm = fm[:, :HF] + fm[:, H - 1 : HF - 1 : -1]
    xmm = fm[:, :HF] - fm[:, H - 1 : HF - 1 : -1]
    xs = np.stack([xpp, xpm, xmp, xmm], axis=1)  # [B*C, 4, n, m]
    xt = xs.transpose(0, 1, 3, 2)  # lhsT: [B*C, 4, m, n]
    xt = xt.reshape(B * C, 4, KT, P, HF).transpose(0, 3, 1, 2, 4)
    return np.ascontiguousarray(xt.reshape(B * C, P, 4 * KT * HF)).astype(BF16)


def _build_program():
    nc = bacc.Bacc(None, target_bir_lowering=False, debug=False)

    x_ext = nc.dram_tensor("x", [SLICES, P, 4 * KT * HF], DT, kind="ExternalInput")
    mt_ext = nc.dram_tensor("mt", [P, 2 * KT * QF], DT, kind="ExternalInput")
    out_ext = nc.dram_tensor("out", [SLICES, 4, QF, QF], DT, kind="ExternalOutput")

    with tile.TileContext(nc) as tc:
        with (
            tc.tile_pool(name="const", bufs=1) as const_pool,
            tc.tile_pool(name="xp", bufs=3) as x_pool,
            tc.tile_pool(name="rp", bufs=2) as r_pool,
            tc.tile_pool(name="op", bufs=8) as o_pool,
            tc.tile_pool(name="ps", bufs=8, space="PSUM") as psum_pool,
        ):
            # PE warmup: dummy matmuls keep the tensor engine busy while the
            # first loads land, so the HAM clock gate is already at 2.4 GHz.
            warm_f32 = const_pool.tile([P, QF], mybir.dt.float32, name="warm_f32")
            nc.gpsimd.memset(warm_f32[:], 0.0)
            warm_w = const_pool.tile([P, P], DT, name="warm_w")
            warm_m = const_pool.tile([P, QF], DT, name="warm_m")
            nc.vector.tensor_copy(warm_w[:], warm_f32[:, :P])
            nc.vector.tensor_copy(warm_m[:], warm_f32[:])
            warm_ps = psum_pool.tile([P, QF], mybir.dt.float32, tag="ps", name="warm_ps")
            for _ in range(36):
                nc.tensor.matmul(warm_ps[:], warm_w[:], warm_m[:], start=True, stop=True)

            # Constants: MPt (c=0) / MMt (c=1), [128, c, kl, j]
            mt_dram = mt_ext[:].rearrange("p (c k j) -> p c k j", c=2, k=KT)
            mt_sb = const_pool.tile([P, 2, KT, QF], DT, name="mt")
            nc.sync.dma_start(mt_sb[:], mt_dram[:])

            x_dram = x_ext[:].rearrange("s p (t k n) -> s p t k n", t=4, k=KT)
            # DRAM-side view for paired quadrant stores: [s, i=mi*120+p, q, j]
            out_q = out_ext[:].rearrange("s q (mi p) j -> s mi p q j", p=M2)

            for s in range(SLICES):
                x_sb = x_pool.tile([P, 4, KT, HF], DT, tag="x")
                nc.sync.dma_start(x_sb[:], x_dram[s])

                # Stage 1: P_t = x_t @ {MP,MM}^T, combined in pairs into
                # r_sb[:, u]: u=0:A=P1+P2, u=1:C=P1-P2, u=2:B=P3+P4, u=3:D=P3-P4
                # stored K-striped for stage 2: r_sb[p, u, nt, j] = U[nt*128+p, j]
                r_sb = r_pool.tile([P, 4, KT, QF], DT, tag="r")
                for pair in range(2):
                    for nt in range(NT1):
                        ps_a = psum_pool.tile([P, QF], mybir.dt.float32, tag="ps")
                        ps_b = psum_pool.tile([P, QF], mybir.dt.float32, tag="ps")
                        for kl in range(KT):
                            nc.tensor.matmul(
                                ps_a[:],
                                x_sb[:, 2 * pair, kl, nt * P : (nt + 1) * P],
                                mt_sb[:, 0, kl, :],
                                start=(kl == 0),
                                stop=(kl == KT - 1),
                            )
                        for kl in range(KT):
                            nc.tensor.matmul(
                                ps_b[:],
                                x_sb[:, 2 * pair + 1, kl, nt * P : (nt + 1) * P],
                                mt_sb[:, 1, kl, :],
                                start=(kl == 0),
                                stop=(kl == KT - 1),
                            )
                        # DVE has a single PSUM read port: drain ps_b via
                        # ScalarE first, then combine PSUM + SBUF on DVE.
                        pb_sb = o_pool.tile([P, QF], mybir.dt.float32, tag="o")
                        nc.scalar.copy(pb_sb[:], ps_b[:])
                        nc.vector.tensor_add(
                            out=r_sb[:, 2 * pair, nt, :], in0=ps_a[:], in1=pb_sb[:]
                        )
                        nc.vector.tensor_sub(
                            out=r_sb[:, 2 * pair + 1, nt, :], in0=ps_a[:], in1=pb_sb[:]
                        )

                # Stage 2: qp=0: Q1=MP A, Q2=MM B -> t_tl, t_bl (quads 0,1)
                #          qp=1: Q3=MP C, Q4=MM D -> t_tr, t_br (quads 2,3)
                for qp in range(2):
                    for mi in range(MT2):
                        ps_1 = psum_pool.tile([P, QF], mybir.dt.float32, tag="ps")
                        ps_2 = psum_pool.tile([P, QF], mybir.dt.float32, tag="ps")
                        for kl in range(KT):
                            nc.tensor.matmul(
                                ps_1[:M2, :],
                                mt_sb[:, 0, kl, mi * M2 : (mi + 1) * M2],
                                r_sb[:, qp, kl, :],
                                start=(kl == 0),
                                stop=(kl == KT - 1),
                            )
                        for kl in range(KT):
                            nc.tensor.matmul(
                                ps_2[:M2, :],
                                mt_sb[:, 1, kl, mi * M2 : (mi + 1) * M2],
                                r_sb[:, 2 + qp, kl, :],
                                start=(kl == 0),
                                stop=(kl == KT - 1),
                            )
                        # ScalarE drains PSUM to SBUF bf16; DVE then combines
                        # in the 2x packed mode (all-bf16, all-SBUF).
                        q1c = o_pool.tile([M2, QF], DT, tag="o")
                        q2c = o_pool.tile([M2, QF], DT, tag="o")
                        nc.scalar.copy(q1c[:], ps_1[:M2, :])
                        nc.scalar.copy(q2c[:], ps_2[:M2, :])
                        o2 = o_pool.tile([M2, 2, QF], DT, tag="o")
                        nc.vector.tensor_add(out=o2[:, 0, :], in0=q1c[:], in1=q2c[:])
                        nc.vector.tensor_sub(out=o2[:, 1, :], in0=q1c[:], in1=q2c[:])
                        nc.sync.dma_start(
                            out_q[s, mi, :, 2 * qp : 2 * qp + 2, :], o2[:]
                        )

    nc.compile()
    return nc


_CACHE: dict = {}


def _get_program():
    if "nc" not in _CACHE:
        _CACHE["nc"] = _build_program()
        _CACHE["mt"] = _build_consts()
    return _CACHE["nc"], _CACHE["mt"]


def kernel(x: np.ndarray, _trace: bool = False):
    assert x.shape == (B, C, H, H), x.shape
    nc, mt_arr = _get_program()
    x = np.ascontiguousarray(x, dtype=np.float32)
    x_arr = _fold_inputs(x)
    per_core = B // N_CORES
    in_maps = [
        {"x": x_arr[i * SLICES : (i + 1) * SLICES], "mt": mt_arr}
        for i in range(N_CORES)
    ]
    res = run_bass_kernel_spmd(nc, in_maps, list(range(N_CORES)), trace=_trace)
    out = np.empty((B, C, OUT, OUT), dtype=np.float32)
    for i in range(N_CORES):
        q = np.asarray(res.results[i]["out"]).astype(np.float32)
        q = q.reshape(per_core, C, 4, QF, QF)
        blk = out[i * per_core : (i + 1) * per_core]
        blk[:, :, :QF, :QF] = q[:, :, 0]
        blk[:, :, QF:, :QF] = q[:, :, 1, ::-1, :]
        blk[:, :, :QF, QF:] = q[:, :, 2, :, ::-1]
        blk[:, :, QF:, QF:] = q[:, :, 3, ::-1, ::-1]
    if _trace:
        return out, res
    return out
